# revision 4
# baseline (speedup 1.0000x reference)
"""Multi-head attention (B=2, S=2048, D=1024, H=16) on 8 Trainium2 cores, v2.

Sharding as v1: head-group parallel (2 heads/core) for QKV + attention; one
8-rank AllToAll per batch redistributes context to token-sharding; each core
runs the output projection (full W_o) for its 2x256-token chunk.

v2 restructures attention around the cost model:
  - scores per k-tile with BOTH heads fused in one PSUM tile / one exp
    instruction (fewer, larger ACT ops; ACT is the attention-phase limiter)
  - AV with exp-weights STATIONARY and V moving (stationary loads are free,
    so AV moving-column count halves vs v1); softmax denominators still come
    free via the ones-column in V
  - softmax normalize via per-partition reciprocal + tensor_scalar (denoms
    land on partitions now); context is transposed back to feature-major
    with one dma_start_transpose per (batch, q-tile) on the DMA engine
  - QKV projection of chunk j+1 is interleaved into attention of q-tile j,
    so the tensor engine fills ACT-latency bubbles with real work and each
    batch's attention (and its AllToAll) starts as early as possible

Per-core layouts (features on partitions, "transposed"):
  xt  [D=1024, T=4096]  bf16  X^T, replicated
  wq/wk/wv [128, 8*128] bf16  core's 2-head column slice, SBUF layout
  wo  [128, 8*1024]     bf16  replicated, same SBUF layout
  bo  [128, 8]          f32   replicated, partition-major
  msk [128, 1408]       bf16  packed causal masks for diagonal k-tile
                              offsets 0..3 (widths 512/384/256/128)
  out [D, 512]          f32   out^T: cols 0:256 batch-0 tokens
                              [c*256,(c+1)*256), cols 256:512 same of batch 1
"""

import os
import sys
from contextlib import ExitStack

for _p in ("/opt/trn_rl_repo",):
    if os.path.isdir(_p) and _p not in sys.path:
        sys.path.insert(0, _p)

import numpy as np
import ml_dtypes

import concourse.bass as bass
import concourse.tile as tile
from concourse import bacc, mybir
from concourse.bass import ts
from concourse.bass_utils import run_bass_kernel_spmd

BF16 = ml_dtypes.bfloat16
BF = mybir.dt.bfloat16
F32 = mybir.dt.float32

B, S, D, H, DH = 2, 2048, 1024, 16, 64
NCORES = 8
T = B * S              # 4096 flattened tokens
FPC = D // NCORES      # 128 features per core (2 heads)
CH2 = S // NCORES      # 256 tokens per (core, batch) in the output phase
DT = D // 128          # 8 contraction tiles over D
QT = 512               # attention q-tile
NQ = S // QT           # 4 q-tiles per batch
NKT = S // 128         # 16 k-tiles per batch
MW = [QT, QT - 128, QT - 256, QT - 384]  # packed mask widths, o=0..3
MOFF = [0, QT, QT + 384, QT + 640]       # block offsets
MTOT = QT + 768 + 128  # packed masks + 128x128 identity (PE transpose)

_BUILD_CACHE = {}


def _build(amp=1, collective=True, num_devices=NCORES, compile=True):
    key = (amp, collective, num_devices, compile)
    if key in _BUILD_CACHE:
        return _BUILD_CACHE[key]
    nc = bacc.Bacc("TRN2", target_bir_lowering=False, debug=False,
                   num_devices=num_devices)
    xt = nc.dram_tensor("xt", [D, T], BF, kind="ExternalInput").ap()
    wq = nc.dram_tensor("wq", [128, DT * FPC], BF, kind="ExternalInput").ap()
    wk = nc.dram_tensor("wk", [128, DT * FPC], BF, kind="ExternalInput").ap()
    wv = nc.dram_tensor("wv", [128, DT * FPC], BF, kind="ExternalInput").ap()
    wo = nc.dram_tensor("wo", [128, DT * D], BF, kind="ExternalInput").ap()
    bo = nc.dram_tensor("bo", [128, DT], F32, kind="ExternalInput").ap()
    msk = nc.dram_tensor("msk", [128, MTOT], BF, kind="ExternalInput").ap()
    out = nc.dram_tensor("out", [D, 2 * CH2], F32, kind="ExternalOutput").ap()

    with tile.TileContext(nc) as tc, ExitStack() as ctx:
        pers = ctx.enter_context(tc.tile_pool(name="pers", bufs=1))
        # PSUM (8 banks of 2KB/partition):
        #   sc  [128,1024] f32 = 2 banks x2 bufs = 4  (scores; also outproj)
        #   av  [128, 512] f32 = 1 bank  x2 bufs = 2  (AV accum, one per head)
        #   qk  [128, 512] f32 = 1 bank  x2 bufs = 2  (QKV proj, junk)
        ps = ctx.enter_context(tc.tile_pool(name="ps", bufs=2, space="PSUM"))
        work = ctx.enter_context(tc.tile_pool(name="work", bufs=4))
        sm = ctx.enter_context(tc.tile_pool(name="sm", bufs=4))
        dram = ctx.enter_context(tc.tile_pool(name="dram", bufs=1, space="DRAM"))

        # ---- persistent SBUF tensors
        xts = pers.tile([128, DT * T], BF, tag="xts")
        xts_d = [xts[:, d * T:(d + 1) * T] for d in range(DT)]
        qts = pers.tile([128, T], BF, tag="qts")
        kts = pers.tile([128, T], BF, tag="kts")
        # Vn per (batch, k-tile): [128 tokens, 130] = [V_h0 | 1 | V_h1 | 1]
        vns = pers.tile([128, B * NKT * 130], BF, tag="vns")
        wqs = pers.tile([128, DT * FPC], BF, tag="wqs")
        wks = pers.tile([128, DT * FPC], BF, tag="wks")
        wvs = pers.tile([128, DT * FPC], BF, tag="wvs")
        wos = pers.tile([128, DT * D], BF, tag="wos")
        mks = pers.tile([128, MTOT], BF, tag="mks")
        bos = pers.tile([128, DT], F32, tag="bos")
        a2s = [pers.tile([128, NCORES * CH2], BF, tag=f"a2s{b}",
                         name=f"a2s{b}") for b in range(B)]
        ots = [pers.tile([128, DT * CH2], F32, tag=f"ots{b}",
                         name=f"ots{b}") for b in range(B)]

        def vn_ap(b, hl, t):
            o = (b * NKT + t) * 130 + hl * 65
            return vns[:, o:o + 65]

        # ---- load weights / constants (host-prepped SBUF layouts); X^T goes
        # chunk-major (one 1MB DMA per 512-token chunk covering all d-tiles).
        # All big input loads go on the ACT hwdge queue: the SP queue is FIFO
        # and belongs to attention's transpose/store chain, which must not
        # wait behind ~30µs of input transfers.
        xts_3d = xts[:].rearrange("p (d t) -> p d t", t=T)
        xt_3d = xt.rearrange("(d p) t -> p d t", p=128)
        nc.sync.dma_start(wqs[:], wq[:])
        nc.sync.dma_start(xts_3d[:, :, ts(0, T // 16)],
                          xt_3d[:, :, ts(0, T // 16)])
        nc.sync.dma_start(xts_3d[:, :, ts(1, T // 16)],
                          xt_3d[:, :, ts(1, T // 16)])
        nc.sync.dma_start(wks[:], wk[:])
        nc.sync.dma_start(wvs[:], wv[:])
        nc.sync.dma_start(mks[:], msk[:])
        nc.sync.dma_start(bos[:], bo[:])
        nc.sync.dma_start(xts_3d[:, :, ts(2, T // 16)],
                          xt_3d[:, :, ts(2, T // 16)])
        nc.sync.dma_start(xts_3d[:, :, ts(3, T // 16)],
                          xt_3d[:, :, ts(3, T // 16)])
        for c8 in range(2, 8):
            nc.sync.dma_start(xts_3d[:, :, ts(c8, T // 8)],
                              xt_3d[:, :, ts(c8, T // 8)])
        nc.sync.dma_start(wos[:], wo[:])
        nc.vector.memset(
            vns[:].rearrange("p (n c) -> p n c", c=65)[:, :, 64:65], 1.0)

        # PE pre-warm: dependency-free matmuls on a never-written junk tile
        # release the HAM clock gate before the first real projection lands
        junk = pers.tile([128, 640], BF, tag="junk")
        nc.vector.memset(junk[:], 0.0)

        def junk_mm(w=512):
            jt = ps.tile([128, 512], F32, tag="qk", name="jt")
            nc.tensor.matmul(jt[:, 0:w], junk[:, 0:128], junk[:, 128:128 + w],
                             start=True, stop=True)

        for _ in range(10):
            junk_mm()

        # token-major slots [tok, feat]: the send side needs no transpose
        # (attention q-tiles finish while a collective may be in flight, and
        # DmaTransposeAnt serializes against collectives); each receiver
        # transposes per-peer slot back to feature-major instead.
        a2a_in = [dram.tile([NCORES, CH2, FPC], BF, tag=f"a2a_in{b}",
                            name=f"a2a_in{b}") for b in range(B)]
        a2a_out = [dram.tile([NCORES, CH2, FPC], BF, tag=f"a2a_out{b}",
                             name=f"a2a_out{b}") for b in range(B)]

        # ---- QKV projection units. Emission order IS scheduler priority:
        # units are emitted right after the attention q-tile that precedes
        # their first use, and the Tile scheduler pulls them forward into
        # PE bubbles (exp-latency waits) by data-readiness.

        def qk_unit(wsb, dst, off, w=512):
            pt = ps.tile([128, 512], F32, tag="qk", name="pt")
            for d in range(DT):
                nc.tensor.matmul(
                    pt[:, 0:w], wsb[:, ts(d, FPC)],
                    xts_d[d][:, off:off + w],
                    start=(d == 0), stop=(d == DT - 1))
            nc.vector.tensor_copy(dst[:, off:off + w], pt[:, 0:w])

        def v_unit(b, tt):
            tg = b * NKT + tt
            pvt = ps.tile([128, 512], F32, tag="qk", name="pvt")
            for d in range(DT):
                nc.tensor.matmul(
                    pvt[:, 0:FPC], xts_d[d][:, ts(tg, 128)],
                    wvs[:, ts(d, FPC)],
                    start=(d == 0), stop=(d == DT - 1))
            o = (b * NKT + tt) * 130
            nc.vector.tensor_copy(
                vns[:, o:o + 130].rearrange(
                    "p (h c) -> p h c", c=65)[:, :, 0:DH],
                pvt[:, 0:FPC].rearrange("p (h c) -> p h c", c=DH))

        def emit_chunk(b, ch, split=False):
            off = b * S + ch * 512
            if split:
                for wsb, dst in ((wqs, qts), (wks, kts)):
                    for o2 in (0, 256):
                        qk_unit(wsb, dst, off + o2, 256)
            else:
                qk_unit(wqs, qts, off)
                qk_unit(wks, kts, off)
            for tt in range(4 * ch, 4 * ch + 4):
                v_unit(b, tt)

        def attention_j(b, j):
            """Attention for batch b, q-tile j (both heads), per k-tile:
            scores(t) -> exp(t) -> mask -> AV(t). The scheduler overlaps
            tiles and pulls QKV filler units into exp-latency bubbles."""
            qoff = b * S
            nkt = (j + 1) * 4
            avh = [ps.tile([128, 512], F32, tag="av", name=f"av{b}{j}{h}")
                   for h in range(2)]
            for t in range(nkt):
                o = t - 4 * j  # >= 0 on diagonal tiles
                vs = max(o, 0) * 128
                w = 512 - vs
                # head h at fixed column h*512 so each head's scores stay
                # inside one PSUM bank even when the diagonal trims w < 512
                sct = ps.tile([128, 1024], F32, tag="sc")
                for h in range(2):
                    nc.tensor.matmul(
                        sct[:, h * 512:h * 512 + w],
                        kts[h * DH:(h + 1) * DH,
                            qoff + t * 128:qoff + t * 128 + 128],
                        qts[h * DH:(h + 1) * DH,
                            qoff + j * QT + vs:qoff + (j + 1) * QT],
                        start=True, stop=True)
                ext = work.tile([128, 1024], BF, tag="exp")
                sct_3d = sct[:].rearrange("p (h c) -> p h c", h=2)[:, :, 0:w]
                ext_3d = ext[:].rearrange("p (h c) -> p h c", h=2)[:, :, 0:w]
                nc.scalar.activation(
                    ext_3d, sct_3d,
                    mybir.ActivationFunctionType.Exp,
                    scale=float(1.0 / np.sqrt(DH)))
                if o >= 0:
                    for h in range(2):
                        nc.vector.tensor_mul(
                            ext[:, h * 512:h * 512 + w],
                            ext[:, h * 512:h * 512 + w],
                            mks[:, MOFF[o]:MOFF[o] + w])
                for h in range(2):
                    for s_ in range(4):
                        if s_ * 128 < vs:
                            continue
                        nc.tensor.matmul(
                            avh[h][:, s_ * 65:s_ * 65 + 65],
                            ext[:, h * 512 + s_ * 128 - vs:
                                h * 512 + s_ * 128 - vs + 128],
                            vn_ap(b, h, t),
                            start=(t == 0 and s_ == 0),
                            stop=(t == nkt - 1 and s_ == 3))

            # normalize: per-partition denominators -> reciprocal ->
            # tensor_scalar. ctxn columns pack (s, h, f) so each token-major
            # peer store below is a clean 3D access pattern.
            ctxn = sm.tile([128, 512], BF, tag="ctxn")
            for h in range(2):
                rc = sm.tile([128, 4], F32, tag="rc")
                nc.vector.reciprocal(
                    rc[:].rearrange("p (s o) -> p s o", o=1),
                    avh[h][:, 0:260].rearrange(
                        "p (s c) -> p s c", c=65)[:, :, 64:65])
                for s_ in range(4):
                    co = s_ * 128 + h * 64
                    nc.vector.tensor_scalar_mul(
                        ctxn[:, co:co + 64],
                        avh[h][:, s_ * 65:s_ * 65 + 64],
                        rc[:, s_:s_ + 1])
            # token-major slot [tok, feat]: dst[s0*128+q, h*64+f]
            for s1 in range(2):
                nc.sync.dma_start(
                    a2a_in[b][2 * j + s1].rearrange(
                        "(s0 q) f -> q s0 f", s0=2),
                    ctxn[:].rearrange("q (s1 s0 f) -> q s0 f s1",
                                      s1=2, s0=2)[:, :, :, s1])

        def a2a(b):
            if collective:
                nc.gpsimd.collective_compute(
                    "AllToAll", mybir.AluOpType.bypass,
                    replica_groups=[list(range(NCORES))],
                    ins=[a2a_in[b].opt()], outs=[a2a_out[b].opt()])
            else:
                nc.sync.dma_start(a2a_out[b][:], a2a_in[b][:])
            # transpose per peer slot while loading, so the projection's
            # first peer-matmuls start right after the first peer lands
            for jb in range(NCORES):
                nc.sync.dma_start_transpose(
                    a2s[b][:, jb * CH2:(jb + 1) * CH2], a2a_out[b][jb])

        def out_proj1():
            # peer-outer: each arriving a2s[1] peer slice feeds one matmul
            # into each of 8 f-accumulators (both "sc" slots hold 4 regions
            # each), so the projection tracks the receive transposes and
            # completes right after the last peer lands.
            po = [ps.tile([128, 1024], F32, tag="sc", name=f"po{i}")
                  for i in range(2)]
            for jb in range(NCORES):
                for f in range(DT):
                    nc.tensor.matmul(
                        po[f // 4][:, (f % 4) * CH2:(f % 4 + 1) * CH2],
                        wos[:, jb * D + f * 128: jb * D + (f + 1) * 128],
                        a2s[1][:, ts(jb, CH2)],
                        start=(jb == 0 and f % 4 == 0),
                        stop=(jb == NCORES - 1 and f % 4 == 3))
            for f in range(DT):
                nc.vector.tensor_scalar_add(
                    ots[1][:, ts(f, CH2)],
                    po[f // 4][:, (f % 4) * CH2:(f % 4 + 1) * CH2],
                    bos[:, f:f + 1])
                if f % 2 == 1:
                    nc.scalar.dma_start(
                        out.rearrange("(g f p) c -> p g f c",
                                      p=128, g=4)[:, f // 2, :, ts(1, CH2)],
                        ots[1][:].rearrange("p (g f c) -> p g f c",
                                            c=CH2, g=4)[:, f // 2])

        def out_proj(b, store=True, nf=DT):
            # output projection for this core's batch-b 256-token chunk;
            # store=False re-runs are idempotent PE filler that keeps the
            # array warm across the batch-1 redistribute latency
            for f in range(nf):
                pot_full = ps.tile([128, 1024], F32, tag="sc")
                pot = pot_full[:, 0:CH2]
                for jb in range(NCORES):
                    nc.tensor.matmul(
                        pot[:], wos[:, jb * D + f * 128: jb * D + (f + 1) * 128],
                        a2s[b][:, ts(jb, CH2)],
                        start=(jb == 0), stop=(jb == NCORES - 1))
                if store:
                    nc.vector.tensor_scalar_add(
                        ots[b][:, ts(f, CH2)], pot[:], bos[:, f:f + 1])
                if store and f in (1, 3, 5):
                    # store per f-pair so stores overlap later matmuls
                    h = f // 2
                    nc.scalar.dma_start(
                        out.rearrange("(g f p) c -> p g f c",
                                      p=128, g=4)[:, h, :, ts(b, CH2)],
                        ots[b][:].rearrange("p (g f c) -> p g f c",
                                            c=CH2, g=4)[:, h])
                elif store and f >= 6:
                    nc.scalar.dma_start(out[ts(f, 128), ts(b, CH2)],
                                        ots[b][:, ts(f, CH2)])

        for _rep in range(amp):
            # attention of q-tile j is emitted (= prioritized) before the
            # QKV chunk units it does not yet need; the scheduler interleaves
            # by readiness. First A2A overlaps batch-1 work; out_proj(0) and
            # idempotent filler passes cover the second.
            emit_chunk(0, 0, split=True)
            for j in range(NQ):
                attention_j(0, j)
                if j < NQ - 1:
                    emit_chunk(0, j + 1)
                else:
                    emit_chunk(1, 0)
            for j in range(NQ):
                attention_j(1, j)
                if j == 1:
                    a2a(0)
                if j < NQ - 1:
                    emit_chunk(1, j + 1)
            a2a(1)
            out_proj(0)
            out_proj(0, store=False)
            out_proj(0, store=False)
            out_proj(0, store=False)
            out_proj1()

        if os.environ.get("BASS_MHA_DEBUG", "0") == "1":
            dbg = {"dbg_q": qts, "dbg_k": kts, "dbg_v": vns,
                   "dbg_a2s0": a2s[0], "dbg_a2s1": a2s[1]}
            for nm, t in dbg.items():
                dt_ = nc.dram_tensor(nm, list(t.shape), t.dtype,
                                     kind="ExternalOutput").ap()
                nc.sync.dma_start(dt_, t[:])
            for b in range(B):
                dt_ = nc.dram_tensor(f"dbg_ain{b}", [NCORES, FPC, CH2],
                                     BF, kind="ExternalOutput").ap()
                nc.sync.dma_start(dt_, a2a_in[b][:])

    if compile:
        nc.compile()
    _BUILD_CACHE[key] = nc
    return nc


def _sbuf_layout(w):
    # [D, F] -> [128, DT*F]: partition p holds rows {d*128+p}, d-major cols
    d, f = w.shape
    return np.ascontiguousarray(
        w.reshape(d // 128, 128, f).transpose(1, 0, 2).reshape(128, -1))


def _make_inputs(X, W_q, W_k, W_v, W_o, b_o):
    Xf = np.asarray(X, np.float32).reshape(T, D)
    xt = np.ascontiguousarray(Xf.T).astype(BF16)
    wo = _sbuf_layout(np.asarray(W_o, np.float32)).astype(BF16)
    bo = np.ascontiguousarray(
        np.asarray(b_o, np.float32).reshape(DT, 128).T)
    kk = np.arange(128)[:, None]
    msk = np.concatenate(
        [(np.arange(MW[o])[None, :] >= kk) for o in range(4)]
        + [np.eye(128)], axis=1).astype(BF16)
    in_maps = []
    for c in range(NCORES):
        sl = slice(c * FPC, (c + 1) * FPC)
        in_maps.append({
            "xt": xt,
            "wq": _sbuf_layout(np.asarray(W_q, np.float32)[:, sl]).astype(BF16),
            "wk": _sbuf_layout(np.asarray(W_k, np.float32)[:, sl]).astype(BF16),
            "wv": _sbuf_layout(np.asarray(W_v, np.float32)[:, sl]).astype(BF16),
            "wo": wo,
            "bo": bo,
            "msk": msk,
        })
    return in_maps


def kernel(X, W_q, W_k, W_v, W_o, b_o):
    nc = _build()
    in_maps = _make_inputs(X, W_q, W_k, W_v, W_o, b_o)
    res = run_bass_kernel_spmd(nc, in_maps, list(range(NCORES)))
    out_t = np.empty((D, T), np.float32)
    for c in range(NCORES):
        o = res.results[c]["out"]  # [D, 512]
        out_t[:, c * CH2:(c + 1) * CH2] = o[:, 0:CH2]
        out_t[:, S + c * CH2:S + (c + 1) * CH2] = o[:, CH2:2 * CH2]
    return np.ascontiguousarray(out_t.T).reshape(B, S, D).astype(np.float32)


# revision 5
# speedup vs baseline: 1.0227x; 1.0227x over previous
"""Multi-head attention (B=2, S=2048, D=1024, H=16) on 8 Trainium2 cores, v2.

Sharding as v1: head-group parallel (2 heads/core) for QKV + attention; one
8-rank AllToAll per batch redistributes context to token-sharding; each core
runs the output projection (full W_o) for its 2x256-token chunk.

v2 restructures attention around the cost model:
  - scores per k-tile with BOTH heads fused in one PSUM tile / one exp
    instruction (fewer, larger ACT ops; ACT is the attention-phase limiter)
  - AV with exp-weights STATIONARY and V moving (stationary loads are free,
    so AV moving-column count halves vs v1); softmax denominators still come
    free via the ones-column in V
  - softmax normalize via per-partition reciprocal + tensor_scalar (denoms
    land on partitions now); context is transposed back to feature-major
    with one dma_start_transpose per (batch, q-tile) on the DMA engine
  - QKV projection of chunk j+1 is interleaved into attention of q-tile j,
    so the tensor engine fills ACT-latency bubbles with real work and each
    batch's attention (and its AllToAll) starts as early as possible

Per-core layouts (features on partitions, "transposed"):
  xt  [D=1024, T=4096]  bf16  X^T, replicated
  wq/wk/wv [128, 8*128] bf16  core's 2-head column slice, SBUF layout
  wo  [128, 8*1024]     bf16  replicated, same SBUF layout
  bo  [128, 8]          f32   replicated, partition-major
  msk [128, 1408]       bf16  packed causal masks for diagonal k-tile
                              offsets 0..3 (widths 512/384/256/128)
  out [D, 512]          f32   out^T: cols 0:256 batch-0 tokens
                              [c*256,(c+1)*256), cols 256:512 same of batch 1
"""

import os
import sys
from contextlib import ExitStack

for _p in ("/opt/trn_rl_repo",):
    if os.path.isdir(_p) and _p not in sys.path:
        sys.path.insert(0, _p)

import numpy as np
import ml_dtypes

import concourse.bass as bass
import concourse.tile as tile
from concourse import bacc, mybir
from concourse.bass import ts
from concourse.bass_utils import run_bass_kernel_spmd

BF16 = ml_dtypes.bfloat16
BF = mybir.dt.bfloat16
F32 = mybir.dt.float32

B, S, D, H, DH = 2, 2048, 1024, 16, 64
NCORES = 8
T = B * S              # 4096 flattened tokens
FPC = D // NCORES      # 128 features per core (2 heads)
CH2 = S // NCORES      # 256 tokens per (core, batch) in the output phase
DT = D // 128          # 8 contraction tiles over D
QT = 512               # attention q-tile
NQ = S // QT           # 4 q-tiles per batch
NKT = S // 128         # 16 k-tiles per batch
MW = [QT, QT - 128, QT - 256, QT - 384]  # packed mask widths, o=0..3
MOFF = [0, QT, QT + 384, QT + 640]       # block offsets
MTOT = QT + 768 + 128  # packed masks + 128x128 identity (PE transpose)

_BUILD_CACHE = {}


def _build(amp=1, collective=True, num_devices=NCORES, compile=True):
    key = (amp, collective, num_devices, compile)
    if key in _BUILD_CACHE:
        return _BUILD_CACHE[key]
    nc = bacc.Bacc("TRN2", target_bir_lowering=False, debug=False,
                   num_devices=num_devices)
    xt = nc.dram_tensor("xt", [D, T], BF, kind="ExternalInput").ap()
    wq = nc.dram_tensor("wq", [128, DT * FPC], BF, kind="ExternalInput").ap()
    wk = nc.dram_tensor("wk", [128, DT * FPC], BF, kind="ExternalInput").ap()
    wv = nc.dram_tensor("wv", [128, DT * FPC], BF, kind="ExternalInput").ap()
    wo = nc.dram_tensor("wo", [128, DT * D], BF, kind="ExternalInput").ap()
    bo = nc.dram_tensor("bo", [128, DT], F32, kind="ExternalInput").ap()
    msk = nc.dram_tensor("msk", [128, MTOT], BF, kind="ExternalInput").ap()
    out = nc.dram_tensor("out", [D, 2 * CH2], F32, kind="ExternalOutput").ap()

    with tile.TileContext(nc) as tc, ExitStack() as ctx:
        pers = ctx.enter_context(tc.tile_pool(name="pers", bufs=1))
        # PSUM (8 banks of 2KB/partition):
        #   sc  [128,1024] f32 = 2 banks x2 bufs = 4  (scores; also outproj)
        #   av  [128, 512] f32 = 1 bank  x2 bufs = 2  (AV accum, one per head)
        #   qk  [128, 512] f32 = 1 bank  x2 bufs = 2  (QKV proj, junk)
        ps = ctx.enter_context(tc.tile_pool(name="ps", bufs=2, space="PSUM"))
        work = ctx.enter_context(tc.tile_pool(name="work", bufs=4))
        sm = ctx.enter_context(tc.tile_pool(name="sm", bufs=4))
        dram = ctx.enter_context(tc.tile_pool(name="dram", bufs=1, space="DRAM"))

        # ---- persistent SBUF tensors
        xts = pers.tile([128, DT * T], BF, tag="xts")
        xts_d = [xts[:, d * T:(d + 1) * T] for d in range(DT)]
        qts = pers.tile([128, T], BF, tag="qts")
        kts = pers.tile([128, T], BF, tag="kts")
        # Vn per (batch, k-tile): [128 tokens, 130] = [V_h0 | 1 | V_h1 | 1]
        vns = pers.tile([128, B * NKT * 130], BF, tag="vns")
        wqs = pers.tile([128, DT * FPC], BF, tag="wqs")
        wks = pers.tile([128, DT * FPC], BF, tag="wks")
        wvs = pers.tile([128, DT * FPC], BF, tag="wvs")
        wos = pers.tile([128, DT * D], BF, tag="wos")
        mks = pers.tile([128, MTOT], BF, tag="mks")
        bos = pers.tile([128, DT], F32, tag="bos")
        a2s = [pers.tile([128, NCORES * CH2], BF, tag=f"a2s{b}",
                         name=f"a2s{b}") for b in range(B)]
        ots = [pers.tile([128, DT * CH2], F32, tag=f"ots{b}",
                         name=f"ots{b}") for b in range(B)]

        def vn_ap(b, hl, t):
            o = (b * NKT + t) * 130 + hl * 65
            return vns[:, o:o + 65]

        # ---- load weights / constants (host-prepped SBUF layouts); X^T goes
        # chunk-major (one 1MB DMA per 512-token chunk covering all d-tiles).
        # All big input loads go on the ACT hwdge queue: the SP queue is FIFO
        # and belongs to attention's transpose/store chain, which must not
        # wait behind ~30µs of input transfers.
        xts_3d = xts[:].rearrange("p (d t) -> p d t", t=T)
        xt_3d = xt.rearrange("(d p) t -> p d t", p=128)
        nc.sync.dma_start(wqs[:], wq[:])
        nc.sync.dma_start(xts_3d[:, :, ts(0, T // 16)],
                          xt_3d[:, :, ts(0, T // 16)])
        nc.sync.dma_start(xts_3d[:, :, ts(1, T // 16)],
                          xt_3d[:, :, ts(1, T // 16)])
        nc.sync.dma_start(wks[:], wk[:])
        nc.sync.dma_start(wvs[:], wv[:])
        nc.sync.dma_start(mks[:], msk[:])
        nc.sync.dma_start(bos[:], bo[:])
        nc.sync.dma_start(xts_3d[:, :, ts(2, T // 16)],
                          xt_3d[:, :, ts(2, T // 16)])
        nc.sync.dma_start(xts_3d[:, :, ts(3, T // 16)],
                          xt_3d[:, :, ts(3, T // 16)])
        for c8 in range(2, 8):
            nc.sync.dma_start(xts_3d[:, :, ts(c8, T // 8)],
                              xt_3d[:, :, ts(c8, T // 8)])
        nc.sync.dma_start(wos[:], wo[:])
        nc.vector.memset(
            vns[:].rearrange("p (n c) -> p n c", c=65)[:, :, 64:65], 1.0)

        # PE pre-warm: dependency-free matmuls on a never-written junk tile
        # release the HAM clock gate before the first real projection lands
        junk = pers.tile([128, 640], BF, tag="junk")
        nc.vector.memset(junk[:], 0.0)

        def junk_mm(w=512):
            jt = ps.tile([128, 512], F32, tag="qk", name="jt")
            nc.tensor.matmul(jt[:, 0:w], junk[:, 0:128], junk[:, 128:128 + w],
                             start=True, stop=True)

        for _ in range(10):
            junk_mm()

        # token-major slots [tok, feat]: the send side needs no transpose
        # (attention q-tiles finish while a collective may be in flight, and
        # DmaTransposeAnt serializes against collectives); each receiver
        # transposes per-peer slot back to feature-major instead.
        a2a_in = [dram.tile([NCORES, CH2, FPC], BF, tag=f"a2a_in{b}",
                            name=f"a2a_in{b}") for b in range(B)]
        a2a_out = [dram.tile([NCORES, CH2, FPC], BF, tag=f"a2a_out{b}",
                             name=f"a2a_out{b}") for b in range(B)]

        # ---- QKV projection units. Emission order IS scheduler priority:
        # units are emitted right after the attention q-tile that precedes
        # their first use, and the Tile scheduler pulls them forward into
        # PE bubbles (exp-latency waits) by data-readiness.

        def qk_unit(wsb, dst, off, w=512):
            pt = ps.tile([128, 512], F32, tag="qk", name="pt")
            for d in range(DT):
                nc.tensor.matmul(
                    pt[:, 0:w], wsb[:, ts(d, FPC)],
                    xts_d[d][:, off:off + w],
                    start=(d == 0), stop=(d == DT - 1))
            nc.vector.tensor_copy(dst[:, off:off + w], pt[:, 0:w])

        def v_unit(b, tt):
            tg = b * NKT + tt
            pvt = ps.tile([128, 512], F32, tag="qk", name="pvt")
            for d in range(DT):
                nc.tensor.matmul(
                    pvt[:, 0:FPC], xts_d[d][:, ts(tg, 128)],
                    wvs[:, ts(d, FPC)],
                    start=(d == 0), stop=(d == DT - 1))
            o = (b * NKT + tt) * 130
            nc.vector.tensor_copy(
                vns[:, o:o + 130].rearrange(
                    "p (h c) -> p h c", c=65)[:, :, 0:DH],
                pvt[:, 0:FPC].rearrange("p (h c) -> p h c", c=DH))

        def emit_chunk(b, ch, split=False):
            off = b * S + ch * 512
            if split:
                for wsb, dst in ((wqs, qts), (wks, kts)):
                    for o2 in (0, 256):
                        qk_unit(wsb, dst, off + o2, 256)
            else:
                qk_unit(wqs, qts, off)
                qk_unit(wks, kts, off)
            for tt in range(4 * ch, 4 * ch + 4):
                v_unit(b, tt)

        def attention_j(b, j):
            """Attention for batch b, q-tile j (both heads), per k-tile:
            scores(t) -> exp(t) -> mask -> AV(t). The scheduler overlaps
            tiles and pulls QKV filler units into exp-latency bubbles."""
            qoff = b * S
            nkt = (j + 1) * 4
            avh = [ps.tile([128, 512], F32, tag="av", name=f"av{b}{j}{h}")
                   for h in range(2)]
            for t in range(nkt):
                o = t - 4 * j  # >= 0 on diagonal tiles
                vs = max(o, 0) * 128
                w = 512 - vs
                # head h at fixed column h*512 so each head's scores stay
                # inside one PSUM bank even when the diagonal trims w < 512
                sct = ps.tile([128, 1024], F32, tag="sc")
                for h in range(2):
                    nc.tensor.matmul(
                        sct[:, h * 512:h * 512 + w],
                        kts[h * DH:(h + 1) * DH,
                            qoff + t * 128:qoff + t * 128 + 128],
                        qts[h * DH:(h + 1) * DH,
                            qoff + j * QT + vs:qoff + (j + 1) * QT],
                        start=True, stop=True)
                ext = work.tile([128, 1024], BF, tag="exp")
                sct_3d = sct[:].rearrange("p (h c) -> p h c", h=2)[:, :, 0:w]
                ext_3d = ext[:].rearrange("p (h c) -> p h c", h=2)[:, :, 0:w]
                nc.scalar.activation(
                    ext_3d, sct_3d,
                    mybir.ActivationFunctionType.Exp,
                    scale=float(1.0 / np.sqrt(DH)))
                if o >= 0:
                    for h in range(2):
                        nc.vector.tensor_mul(
                            ext[:, h * 512:h * 512 + w],
                            ext[:, h * 512:h * 512 + w],
                            mks[:, MOFF[o]:MOFF[o] + w])
                for h in range(2):
                    for s_ in range(4):
                        if s_ * 128 < vs:
                            continue
                        nc.tensor.matmul(
                            avh[h][:, s_ * 65:s_ * 65 + 65],
                            ext[:, h * 512 + s_ * 128 - vs:
                                h * 512 + s_ * 128 - vs + 128],
                            vn_ap(b, h, t),
                            start=(t == 0 and s_ == 0),
                            stop=(t == nkt - 1 and s_ == 3))

            # normalize: per-partition denominators -> reciprocal ->
            # tensor_scalar. ctxn columns pack (s, h, f) so each token-major
            # peer store below is a clean 3D access pattern.
            ctxn = sm.tile([128, 512], BF, tag="ctxn")
            for h in range(2):
                rc = sm.tile([128, 4], F32, tag="rc")
                nc.vector.reciprocal(
                    rc[:].rearrange("p (s o) -> p s o", o=1),
                    avh[h][:, 0:260].rearrange(
                        "p (s c) -> p s c", c=65)[:, :, 64:65])
                for s_ in range(4):
                    co = s_ * 128 + h * 64
                    nc.vector.tensor_scalar_mul(
                        ctxn[:, co:co + 64],
                        avh[h][:, s_ * 65:s_ * 65 + 64],
                        rc[:, s_:s_ + 1])
            # token-major slot [tok, feat]: dst[s0*128+q, h*64+f]
            for s1 in range(2):
                nc.sync.dma_start(
                    a2a_in[b][2 * j + s1].rearrange(
                        "(s0 q) f -> q s0 f", s0=2),
                    ctxn[:].rearrange("q (s1 s0 f) -> q s0 f s1",
                                      s1=2, s0=2)[:, :, :, s1])

        def a2a(b):
            if collective:
                nc.gpsimd.collective_compute(
                    "AllToAll", mybir.AluOpType.bypass,
                    replica_groups=[list(range(NCORES))],
                    ins=[a2a_in[b].opt()], outs=[a2a_out[b].opt()])
            else:
                nc.sync.dma_start(a2a_out[b][:], a2a_in[b][:])
            # transpose per peer slot while loading, so the projection's
            # first peer-matmuls start right after the first peer lands
            for jb in range(NCORES):
                nc.sync.dma_start_transpose(
                    a2s[b][:, jb * CH2:(jb + 1) * CH2], a2a_out[b][jb])

        def out_proj(b, store=True, nf=DT):
            # output projection for this core's batch-b 256-token chunk;
            # store=False re-runs are idempotent PE filler that keeps the
            # array warm across the batch-1 redistribute latency
            for f in range(nf):
                pot_full = ps.tile([128, 1024], F32, tag="sc")
                pot = pot_full[:, 0:CH2]
                for jb in range(NCORES):
                    nc.tensor.matmul(
                        pot[:], wos[:, jb * D + f * 128: jb * D + (f + 1) * 128],
                        a2s[b][:, ts(jb, CH2)],
                        start=(jb == 0), stop=(jb == NCORES - 1))
                if store:
                    nc.vector.tensor_scalar_add(
                        ots[b][:, ts(f, CH2)], pot[:], bos[:, f:f + 1])
                if store and f in (1, 3, 5):
                    # store per f-pair so stores overlap later matmuls
                    h = f // 2
                    nc.scalar.dma_start(
                        out.rearrange("(g f p) c -> p g f c",
                                      p=128, g=4)[:, h, :, ts(b, CH2)],
                        ots[b][:].rearrange("p (g f c) -> p g f c",
                                            c=CH2, g=4)[:, h])
                elif store and f >= 6:
                    nc.scalar.dma_start(out[ts(f, 128), ts(b, CH2)],
                                        ots[b][:, ts(f, CH2)])

        for _rep in range(amp):
            # attention of q-tile j is emitted (= prioritized) before the
            # QKV chunk units it does not yet need; the scheduler interleaves
            # by readiness. First A2A overlaps batch-1 work; out_proj(0) and
            # idempotent filler passes cover the second.
            emit_chunk(0, 0, split=True)
            for j in range(NQ):
                attention_j(0, j)
                if j < NQ - 1:
                    emit_chunk(0, j + 1)
                else:
                    emit_chunk(1, 0)
            for j in range(NQ):
                attention_j(1, j)
                if j == 1:
                    a2a(0)
                if j < NQ - 1:
                    emit_chunk(1, j + 1)
            a2a(1)
            out_proj(0)
            out_proj(0, store=False)
            out_proj(0, store=False)
            out_proj(0, store=False)
            out_proj(0, store=False, nf=2)
            out_proj(1)

        if os.environ.get("BASS_MHA_DEBUG", "0") == "1":
            dbg = {"dbg_q": qts, "dbg_k": kts, "dbg_v": vns,
                   "dbg_a2s0": a2s[0], "dbg_a2s1": a2s[1]}
            for nm, t in dbg.items():
                dt_ = nc.dram_tensor(nm, list(t.shape), t.dtype,
                                     kind="ExternalOutput").ap()
                nc.sync.dma_start(dt_, t[:])
            for b in range(B):
                dt_ = nc.dram_tensor(f"dbg_ain{b}", [NCORES, FPC, CH2],
                                     BF, kind="ExternalOutput").ap()
                nc.sync.dma_start(dt_, a2a_in[b][:])

    if compile:
        nc.compile()
    _BUILD_CACHE[key] = nc
    return nc


def _sbuf_layout(w):
    # [D, F] -> [128, DT*F]: partition p holds rows {d*128+p}, d-major cols
    d, f = w.shape
    return np.ascontiguousarray(
        w.reshape(d // 128, 128, f).transpose(1, 0, 2).reshape(128, -1))


def _make_inputs(X, W_q, W_k, W_v, W_o, b_o):
    Xf = np.asarray(X, np.float32).reshape(T, D)
    xt = np.ascontiguousarray(Xf.T).astype(BF16)
    wo = _sbuf_layout(np.asarray(W_o, np.float32)).astype(BF16)
    bo = np.ascontiguousarray(
        np.asarray(b_o, np.float32).reshape(DT, 128).T)
    kk = np.arange(128)[:, None]
    msk = np.concatenate(
        [(np.arange(MW[o])[None, :] >= kk) for o in range(4)]
        + [np.eye(128)], axis=1).astype(BF16)
    in_maps = []
    for c in range(NCORES):
        sl = slice(c * FPC, (c + 1) * FPC)
        in_maps.append({
            "xt": xt,
            "wq": _sbuf_layout(np.asarray(W_q, np.float32)[:, sl]).astype(BF16),
            "wk": _sbuf_layout(np.asarray(W_k, np.float32)[:, sl]).astype(BF16),
            "wv": _sbuf_layout(np.asarray(W_v, np.float32)[:, sl]).astype(BF16),
            "wo": wo,
            "bo": bo,
            "msk": msk,
        })
    return in_maps


def kernel(X, W_q, W_k, W_v, W_o, b_o):
    nc = _build()
    in_maps = _make_inputs(X, W_q, W_k, W_v, W_o, b_o)
    res = run_bass_kernel_spmd(nc, in_maps, list(range(NCORES)))
    out_t = np.empty((D, T), np.float32)
    for c in range(NCORES):
        o = res.results[c]["out"]  # [D, 512]
        out_t[:, c * CH2:(c + 1) * CH2] = o[:, 0:CH2]
        out_t[:, S + c * CH2:S + (c + 1) * CH2] = o[:, CH2:2 * CH2]
    return np.ascontiguousarray(out_t.T).reshape(B, S, D).astype(np.float32)
